# revision 26
# baseline (speedup 1.0000x reference)
"""GAT temporal encoder (2-layer GAT x 6 timesteps + GRU) on 8 trn2 cores.

Sharding: 1D node partition (1250 nodes/core, padded to 1280 = 10 windows of
128), edges partitioned by destination, sorted by dst, padded so each window
owns exactly K chunks of 128 edges.

Layer 0 is gather-free on device: the host pre-gathers x[src]||x[dst] per
edge slot (feature-major, bf16), and one PE matmul per 128-edge chunk
against [[W0,V0s],[0,V0d]] yields both xw0[src] and the complete attention
logit. Layer 1 dma_gathers rows of the bf16 xw1||s_src||s_dst table by src
(4 SWDGE queues), with per-window s_dst broadcast to edges via a
host-precomputed one-hot S^T matmul. Segment softmax + aggregation run as
one-hot S matmuls accumulated in PSUM per chunk (normalization applied
after aggregation at the node level, which is algebraically identical).
h0 is all-gathered between layers; GRU/LN/node math are batched f32 ops.
"""
import os
import numpy as np
import ml_dtypes

import bass_rust
import concourse.bacc as bacc
import concourse.tile as tile
from concourse import mybir
from concourse.bass_utils import run_bass_kernel_spmd

AF = mybir.ActivationFunctionType
ALU = mybir.AluOpType
F32 = mybir.dt.float32
BF16 = mybir.dt.bfloat16
I16 = mybir.dt.int16
NPBF = ml_dtypes.bfloat16

NCORES = 8
N, T, F, HID, HEADS, E = 10000, 6, 64, 64, 4, 160000
NPC = N // NCORES               # 1250 nodes per core
NWL = (NPC + 127) // 128        # 10 local windows
NL = NWL * 128                  # 1280 padded local nodes
NG = NCORES * NL                # 10240 padded global nodes
C0 = HID // HEADS               # 16 (layer-0 head dim)
C1 = HID                        # 64 (layer-1 head dim)
ROW1 = 384                      # bf16 cols: xw(256) s_src(4) s_dst(4) pad

LAST_EXEC_NS = None


def _split_waits(nc, maxw=1):
    """This container's walrus allows very few sync-wait slots per
    instruction; move excess waits onto preceding same-engine nops."""
    nsplit = 0
    for func in nc.m.functions:
        for bb in func.blocks:
            new = []
            for ins in bb.instructions:
                si = ins.sync_info
                waits = list(si.on_wait) if si is not None and si.on_wait else []
                if len(waits) > maxw:
                    nsplit += 1
                    rest = waits[maxw:]
                    for i in range(0, len(rest), maxw):
                        nop = mybir.InstNoOp(
                            name=f"I-wsplit-{nsplit}-{i}", ins=[], outs=[]
                        )
                        nop.engine = ins.engine
                        nop.sync_info = bass_rust.SyncInfo(
                            on_wait=rest[i : i + maxw], on_update=[]
                        )
                        new.append(nop)
                    si.on_wait = waits[:maxw]
                new.append(ins)
            bb.instructions = new
    return nsplit


def _wrap16(a):
    """flat [n] -> replicated-wrapped [128, n/16] int16 (flat k at [k%16, k//16],
    replicated across the 8 gpsimd cores' partition groups)."""
    n = a.shape[0]
    assert n % 16 == 0
    return np.tile(a.reshape(n // 16, 16).T, (8, 1)).astype(np.int16).copy()


def _host_prep(inputs):
    x = np.asarray(inputs["x"], np.float32)
    ei = np.asarray(inputs["edge_index"])
    W0 = np.asarray(inputs["W0"], np.float32)
    a_src0 = np.asarray(inputs["a_src0"], np.float32)
    a_dst0 = np.asarray(inputs["a_dst0"], np.float32)
    W1 = np.asarray(inputs["W1"], np.float32)
    a_src1 = np.asarray(inputs["a_src1"], np.float32)
    a_dst1 = np.asarray(inputs["a_dst1"], np.float32)

    loops = np.arange(N, dtype=ei.dtype)
    src = np.concatenate([ei[0], loops])
    dst = np.concatenate([ei[1], loops])
    EP = src.shape[0]  # E + N

    core_of = dst // NPC
    per_core = []
    Kmax = 1
    for c in range(NCORES):
        sel = np.nonzero(core_of == c)[0]
        order = np.argsort(dst[sel], kind="stable")
        eids = sel[order]
        dl = (dst[eids] - c * NPC).astype(np.int64)
        w_of = dl // 128
        cnt = np.bincount(w_of, minlength=NWL)
        Kmax = max(Kmax, int(np.ceil(cnt.max() / 128)))
        per_core.append((eids, dl))

    K = Kmax
    NCH = NWL * K
    EB = NCH * 128
    SBS = [8] * (K // 8) + ([K % 8] if K % 8 else [])

    xbf = x.astype(NPBF)        # [N, T, F]
    percore = []
    slot_of = []
    for c in range(NCORES):
        eids, dl = per_core[c]
        srcg = src[eids]
        srcp = ((srcg // NPC) * NL + (srcg % NPC)).astype(np.int64)
        s_src = np.zeros(EB, np.int64)      # padded-global src per slot
        s_orig = np.zeros(EB, np.int64)     # original src node id (pad->0)
        d_orig = np.zeros(EB, np.int64)     # original dst node id (pad->0)
        s_dstw = np.full(EB, -1, np.int64)  # dst-in-window; -1 => pad edge
        slots = np.empty(len(eids), np.int64)
        w_of = dl // 128
        for w in range(NWL):
            idxs = np.nonzero(w_of == w)[0]
            base = w * K * 128
            s_src[base : base + len(idxs)] = srcp[idxs]
            s_orig[base : base + len(idxs)] = srcg[idxs]
            d_orig[base : base + len(idxs)] = c * NPC + dl[idxs]
            s_dstw[base : base + len(idxs)] = dl[idxs] - w * 128
            slots[idxs] = base + np.arange(len(idxs))

        m = {}
        for si_, sbs in enumerate(SBS):
            arr = np.zeros((NWL, 128, sbs * 8), np.int16)
            arl = np.zeros((NWL, 128, sbs * 8), np.int16)
            c0 = sum(SBS[:si_])
            for w in range(NWL):
                lo = (w * K + c0) * 128
                hi = lo + sbs * 128
                arr[w] = _wrap16(s_src[lo:hi])
                dloc = s_dstw[lo:hi].copy()
                dloc[dloc >= 0] += w * 128
                dloc[dloc < 0] = 0
                arl[w] = _wrap16(dloc)
            m[f"srcp{si_}"] = arr
            m[f"dstl{si_}"] = arl
        # one-hot S / S^T per window (bf16)
        dw = s_dstw.reshape(NWL, K, 128)
        S = np.zeros((NWL, 128, K * 128), NPBF)
        St = np.zeros((NWL, 128, K * 128), NPBF)
        for w in range(NWL):
            for k in range(K):
                d = dw[w, k]
                p = np.nonzero(d >= 0)[0]
                S[w, p, k * 128 + d[p]] = 1
                St[w, d[p], k * 128 + p] = 1
        m["S"] = S
        m["St"] = St
        # host-pre-gathered per-edge x: rows 0:64 = x[src]^T, 64:128 = x[dst]^T
        xc = np.empty((T, 128, EB), NPBF)
        xe = xbf[s_orig]            # [EB, T, F]
        xd = xbf[d_orig]
        xc[:, 0:F, :] = xe.transpose(1, 2, 0)
        xc[:, F:2 * F, :] = xd.transpose(1, 2, 0)
        m["xcomb"] = xc
        ln_ = c * NL + np.arange(NL)
        m["locg0"] = _wrap16(ln_[:NL // 2].astype(np.int64))
        m["locg1"] = _wrap16(ln_[NL // 2:].astype(np.int64))
        percore.append(m)
        slot_of.append((eids, slots))

    V0s = np.einsum("khc,hc->kh", W0.reshape(F, HEADS, C0), a_src0)
    V0d = np.einsum("khc,hc->kh", W0.reshape(F, HEADS, C0), a_dst0)
    # [[W0, V0s], [0, V0d]]: [128, 68]
    W0ee = np.zeros((2 * F, HID + 4), np.float32)
    W0ee[0:F, 0:HID] = W0
    W0ee[0:F, HID:HID + 4] = V0s
    W0ee[F:2 * F, HID:HID + 4] = V0d
    V1s = np.einsum("khc,hc->kh", W1.reshape(HID, HEADS, C1), a_src1)
    V1d = np.einsum("khc,hc->kh", W1.reshape(HID, HEADS, C1), a_dst1)
    W1e = np.concatenate([W1, V1s, V1d], axis=1).astype(NPBF)

    rep = lambda v, n: np.broadcast_to(
        np.asarray(v, np.float32).reshape(1, n), (128, n)).copy()
    consts = {
        "W0ee": W0ee.astype(NPBF), "W1e": W1e,
        "b0": rep(inputs["b0"], HID), "b1": rep(inputs["b1"], HID),
        "lng": rep(inputs["ln_g"], HID), "lnb": rep(inputs["ln_b"], HID),
        "WihT": np.asarray(inputs["W_ih"], np.float32).T.copy(),
        "WhhT": np.asarray(inputs["W_hh"], np.float32).T.copy(),
        "bih": rep(inputs["b_ih"], 3 * HID), "bhh": rep(inputs["b_hh"], 3 * HID),
        "ident": np.eye(128, dtype=np.float32),
    }
    aei = np.stack([src, dst]).astype(ei.dtype)
    return dict(K=K, NCH=NCH, EB=EB, consts=consts, percore=percore,
                slot_of=slot_of, aei=aei, EP=EP)


def _build(K):
    NCH = NWL * K
    EB = NCH * 128
    SBS = [8] * (K // 8) + ([K % 8] if K % 8 else [])
    NSB = len(SBS)
    nc = bacc.Bacc(None, num_swdge_queues=4)
    D = {}

    def par(name, shape, dt=F32, out=False):
        D[name] = nc.declare_dram_parameter(name, list(shape), dt, isOutput=out)
        return D[name]

    par("xcomb", [T, 128, EB], BF16)
    par("W0ee", [2 * F, HID + 4], BF16)
    par("W1e", [HID, HEADS * C1 + 8], BF16)
    par("b0", [128, HID]); par("b1", [128, HID])
    par("lng", [128, HID]); par("lnb", [128, HID])
    par("WihT", [HID, 3 * HID]); par("WhhT", [HID, 3 * HID])
    par("bih", [128, 3 * HID]); par("bhh", [128, 3 * HID])
    par("ident", [128, 128])
    for si_, sbs in enumerate(SBS):
        par(f"srcp{si_}", [NWL, 128, sbs * 8], I16)
        par(f"dstl{si_}", [NWL, 128, sbs * 8], I16)
    par("S", [NWL, 128, K * 128], BF16)
    par("St", [NWL, 128, K * 128], BF16)
    par("locg0", [128, NL // 32], I16)
    par("locg1", [128, NL // 32], I16)
    par("hT_out", [NL, HID], out=True)
    par("alpha_out", [128, NCH * 4], out=True)

    xw1_tabs = [nc.dram_tensor(f"xw1_{i}", [NG, ROW1], BF16) for i in range(2)]
    r_tab = nc.dram_tensor("r_tab", [NL, HID], F32)
    h0T_bounce = nc.dram_tensor("h0T_bounce", [HID, NL], BF16)
    h0T_full = nc.dram_tensor("h0T_full", [NCORES * HID, NL], BF16,
                              addr_space="Shared")

    with tile.TileContext(nc) as tc:
        with tc.tile_pool(name="persist", bufs=1) as pp:
            W0ee_t = pp.tile([2 * F, HID + 4], BF16)
            W1e_t = pp.tile([HID, HEADS * C1 + 8], BF16)
            b0_t = pp.tile([128, HID], F32, tag="b0t")
            b1_t = pp.tile([128, HID], F32, tag="b1t")
            lng_t = pp.tile([128, HID], F32, tag="lngt")
            lnb_t = pp.tile([128, HID], F32, tag="lnbt")
            WihT_t = pp.tile([HID, 3 * HID], F32, tag="wiht")
            WhhT_t = pp.tile([HID, 3 * HID], F32, tag="whht")
            bih_t = pp.tile([128, 3 * HID], F32, tag="biht")
            bhh_t = pp.tile([128, 3 * HID], F32, tag="bhht")
            ident_t = pp.tile([128, 128], F32, tag="identt")
            locg0_t = pp.tile([128, NL // 32], I16, tag="locg0t")
            locg1_t = pp.tile([128, NL // 32], I16, tag="locg1t")
            eps_t = pp.tile([128, 1], F32, tag="epst")
            nc.vector.memset(eps_t[:], 1e-5)
            for nm, tl in [("W0ee", W0ee_t), ("W1e", W1e_t), ("b0", b0_t),
                           ("b1", b1_t), ("lng", lng_t), ("lnb", lnb_t),
                           ("WihT", WihT_t), ("WhhT", WhhT_t), ("bih", bih_t),
                           ("bhh", bhh_t), ("ident", ident_t),
                           ("locg0", locg0_t), ("locg1", locg1_t)]:
                nc.sync.dma_start(out=tl[:], in_=D[nm][:])
            srcp_ts, dstl_ts = [], []
            for w in range(NWL):
                rowt = []
                for si_, sbs in enumerate(SBS):
                    tl = pp.tile([128, sbs * 8], I16, tag=f"srcp{w}_{si_}",
                                 name=f"srcp{w}_{si_}")
                    nc.sync.dma_start(out=tl[:], in_=D[f"srcp{si_}"][w])
                    rowt.append(tl)
                srcp_ts.append(rowt)
                rowt = []
                for si_, sbs in enumerate(SBS):
                    tl = pp.tile([128, sbs * 8], I16, tag=f"dstl{w}_{si_}",
                                 name=f"dstl{w}_{si_}")
                    nc.sync.dma_start(out=tl[:], in_=D[f"dstl{si_}"][w])
                    rowt.append(tl)
                dstl_ts.append(rowt)

            EXall = pp.tile([128, NCH, 4], F32)        # t5 L1 ex (for alpha)
            hsT_t = pp.tile([HID, T * NL], F32)        # LN outputs, transposed
            hgru_t = pp.tile([128, NWL, HID], F32, tag="hgrut")

            def node_phase(t, lay, sp, spp, SUMS, h0T):
                """batched across windows; SUMS [128, NWL, 4+XWC] f32."""
                XWC = HID if lay == 0 else HEADS * C1
                CC = C0 if lay == 0 else C1
                R4 = sp.tile([128, NWL, 4], F32, tag="R4")
                nc.vector.tensor_scalar(out=R4[:], in0=SUMS[:, :, 0:4],
                                        scalar1=1e-16, scalar2=None, op0=ALU.add)
                nc.vector.reciprocal(out=R4[:], in_=R4[:])
                if lay == 1 and t == T - 1:
                    RT = sp.tile([128, NWL, HID], F32, tag="RT")
                    nc.vector.memset(RT[:], 0.0)
                    nc.vector.tensor_copy(out=RT[:, :, 0:4], in_=R4[:])
                    nc.sync.dma_start(
                        out=r_tab[:].rearrange("(w p) c -> p w c", p=128),
                        in_=RT[:])
                ON = sp.tile([128, NWL, XWC], F32, tag=f"ON{lay}", bufs=1)
                nc.vector.tensor_tensor(
                    out=ON[:].rearrange("p w (h c) -> p w h c", h=HEADS),
                    in0=SUMS[:, :, 4:4 + XWC].rearrange(
                        "p w (h c) -> p w h c", h=HEADS),
                    in1=R4[:].unsqueeze(-1).to_broadcast((128, NWL, HEADS, CC)),
                    op=ALU.mult)
                if lay == 1:
                    # head mean -> HM [128, NWL, 64]
                    HM = sp.tile([128, NWL, C1], F32, tag="HM")
                    nc.vector.tensor_tensor(out=HM[:], in0=ON[:, :, 0:C1],
                                            in1=ON[:, :, C1:2 * C1], op=ALU.add)
                    nc.vector.tensor_tensor(out=HM[:], in0=HM[:],
                                            in1=ON[:, :, 2 * C1:3 * C1], op=ALU.add)
                    nc.vector.tensor_tensor(out=HM[:], in0=HM[:],
                                            in1=ON[:, :, 3 * C1:4 * C1], op=ALU.add)
                    nc.vector.tensor_scalar(out=HM[:], in0=HM[:], scalar1=0.25,
                                            scalar2=None, op0=ALU.mult)
                    ON = HM
                bias = b0_t if lay == 0 else b1_t
                nc.vector.tensor_tensor(
                    out=ON[:, :, 0:HID], in0=ON[:, :, 0:HID],
                    in1=bias[:].unsqueeze(1).to_broadcast((128, NWL, HID)),
                    op=ALU.add)
                # elu in place
                e1 = sp.tile([128, NWL, HID], F32, tag="elu1", bufs=1)
                nc.vector.tensor_scalar(out=e1[:], in0=ON[:, :, 0:HID],
                                        scalar1=0.0, scalar2=None, op0=ALU.min)
                nc.scalar.activation(out=e1[:], in_=e1[:], func=AF.Exp)
                e2 = sp.tile([128, NWL, HID], F32, tag="elu2", bufs=1)
                nc.scalar.activation(out=e2[:], in_=ON[:, :, 0:HID], func=AF.Relu)
                nc.vector.scalar_tensor_tensor(
                    out=ON[:, :, 0:HID], in0=e1[:], scalar=-1.0, in1=e2[:],
                    op0=ALU.add, op1=ALU.add)
                if lay == 0:
                    for w in range(NWL):
                        tp = spp.tile([HID, 128], F32, tag="tp", bufs=2)
                        nc.tensor.transpose(tp[:], ON[:, w, 0:HID], ident_t[:])
                        nc.vector.tensor_copy(out=h0T[:, w * 128:(w + 1) * 128],
                                              in_=tp[:])
                    return
                # LayerNorm batched over windows
                MU = sp.tile([128, NWL], F32, tag="MU")
                nc.vector.tensor_reduce(out=MU[:], in_=ON[:],
                                        axis=mybir.AxisListType.X, op=ALU.add)
                nc.vector.tensor_scalar(out=MU[:], in0=MU[:], scalar1=1.0 / HID,
                                        scalar2=None, op0=ALU.mult)
                XC = sp.tile([128, NWL, HID], F32, tag="XC", bufs=1)
                nc.vector.tensor_tensor(
                    out=XC[:], in0=ON[:],
                    in1=MU[:].unsqueeze(-1).to_broadcast((128, NWL, HID)),
                    op=ALU.subtract)
                SQ = sp.tile([128, NWL, HID], F32, tag="SQ", bufs=1)
                nc.vector.tensor_tensor(out=SQ[:], in0=XC[:], in1=XC[:],
                                        op=ALU.mult)
                VV = sp.tile([128, NWL], F32, tag="VV")
                nc.vector.tensor_reduce(out=VV[:], in_=SQ[:],
                                        axis=mybir.AxisListType.X, op=ALU.add)
                nc.vector.tensor_scalar(out=VV[:], in0=VV[:], scalar1=1.0 / HID,
                                        scalar2=1e-5, op0=ALU.mult, op1=ALU.add)
                nc.scalar.activation(out=VV[:], in_=VV[:], func=AF.Sqrt)
                nc.vector.reciprocal(out=VV[:], in_=VV[:])
                nc.vector.tensor_tensor(
                    out=XC[:], in0=XC[:],
                    in1=VV[:].unsqueeze(-1).to_broadcast((128, NWL, HID)),
                    op=ALU.mult)
                nc.vector.tensor_tensor(
                    out=XC[:], in0=XC[:],
                    in1=lng_t[:].unsqueeze(1).to_broadcast((128, NWL, HID)),
                    op=ALU.mult)
                nc.vector.tensor_tensor(
                    out=XC[:], in0=XC[:],
                    in1=lnb_t[:].unsqueeze(1).to_broadcast((128, NWL, HID)),
                    op=ALU.add)
                for w in range(NWL):
                    tp = spp.tile([HID, 128], F32, tag="tp", bufs=2)
                    nc.tensor.transpose(tp[:], XC[:, w, :], ident_t[:])
                    nc.vector.tensor_copy(
                        out=hsT_t[:, t * NL + w * 128: t * NL + (w + 1) * 128],
                        in_=tp[:])

            def gat0(t, sp, spp, h0T):
                XWC = HID
                SUMS = sp.tile([128, NWL, 4 + XWC], F32, tag="SUMS0", bufs=1)
                for w in range(NWL):
                    xct = sp.tile([128, K * 128], BF16, tag="xct")
                    nc.sync.dma_start(
                        out=xct[:],
                        in_=D["xcomb"][t][:, w * K * 128:(w + 1) * K * 128])
                    S_w = sp.tile([128, K * 128], BF16, tag="S_w")
                    nc.sync.dma_start(out=S_w[:], in_=D["S"][w])
                    Ge = sp.tile([128, K, 4 + XWC], BF16, tag="Ge")
                    for k in range(K):
                        psx = spp.tile([128, HID + 4], F32, tag="psx")
                        nc.tensor.matmul(
                            psx[:], xct[:, k * 128:(k + 1) * 128], W0ee_t[:],
                            start=True, stop=True)
                        nc.scalar.copy(out=Ge[:, k, :], in_=psx[:])
                    rhs_w = sp.tile([128, K, 4 + XWC], BF16, tag="rhs0")
                    LK = sp.tile([128, K, 4], BF16, tag="LK")
                    nc.vector.scalar_tensor_tensor(
                        out=LK[:], in0=Ge[:, :, XWC:XWC + 4], scalar=0.2,
                        in1=Ge[:, :, XWC:XWC + 4], op0=ALU.mult, op1=ALU.max)
                    EX = sp.tile([128, K, 4], BF16, tag="EX")
                    nc.scalar.activation(out=EX[:], in_=LK[:], func=AF.Exp)
                    nc.scalar.activation(out=rhs_w[:, :, 0:4], in_=LK[:],
                                         func=AF.Exp)
                    nc.vector.tensor_tensor(
                        out=rhs_w[:, :, 4:4 + XWC].rearrange(
                            "p k (h c) -> p k h c", h=HEADS),
                        in0=Ge[:, :, 0:XWC].rearrange(
                            "p k (h c) -> p k h c", h=HEADS),
                        in1=EX[:].unsqueeze(-1).to_broadcast(
                            (128, K, HEADS, C0)),
                        op=ALU.mult)
                    ps = spp.tile([128, 4 + XWC], F32, tag="agg")
                    for k in range(K):
                        nc.tensor.matmul(
                            ps[:], S_w[:, k * 128:(k + 1) * 128], rhs_w[:, k, :],
                            start=(k == 0), stop=(k == K - 1))
                    nc.scalar.copy(out=SUMS[:, w, :], in_=ps[:])
                node_phase(t, 0, sp, spp, SUMS, h0T)

            def gat1(t, sp, spp):
                XWC = HEADS * C1
                tab = xw1_tabs[t % 2]
                sdAll = sp.tile([128, NWL, 128], BF16, tag="sdAll", bufs=1)
                for g, lg in enumerate((locg0_t, locg1_t)):
                    nc.gpsimd.dma_gather(
                        out_ap=sdAll[:, g * 5:(g + 1) * 5, :],
                        in_ap=tab[:, ROW1 - 128:ROW1],
                        idxs_ap=lg[:],
                        num_idxs=640, num_idxs_reg=640,
                        elem_size=128, elem_step=ROW1, queue_num=3)
                SUMS = sp.tile([128, NWL, 4 + XWC], F32, tag="SUMS1", bufs=1)
                for w in range(NWL):
                    G = sp.tile([128, K, ROW1], BF16, tag="G1", bufs=2)
                    c0 = 0
                    for si_, sbs in enumerate(SBS):
                        nc.gpsimd.dma_gather(
                            out_ap=G[:, c0:c0 + sbs, :], in_ap=tab[:],
                            idxs_ap=srcp_ts[w][si_][:],
                            num_idxs=sbs * 128, num_idxs_reg=sbs * 128,
                            elem_size=ROW1, queue_num=(w * NSB + si_) % 3)
                        c0 += sbs
                    S_w = sp.tile([128, K * 128], BF16, tag="S_w")
                    nc.sync.dma_start(out=S_w[:], in_=D["S"][w])
                    St_w = sp.tile([128, K * 128], BF16, tag="St_w")
                    nc.sync.dma_start(out=St_w[:], in_=D["St"][w])
                    ps2 = spp.tile([128, K, 4], F32, tag="mm2")
                    for k in range(K):
                        nc.tensor.matmul(
                            ps2[:, k, :], St_w[:, k * 128:(k + 1) * 128],
                            sdAll[:, w, 4:8], start=True, stop=True)
                    AL = sp.tile([128, K, 4], BF16, tag="AL")
                    nc.vector.tensor_tensor(
                        out=AL[:], in0=G[:, :, XWC:XWC + 4], in1=ps2[:],
                        op=ALU.add)
                    LK = sp.tile([128, K, 4], BF16, tag="LK")
                    nc.vector.scalar_tensor_tensor(
                        out=LK[:], in0=AL[:], scalar=0.2, in1=AL[:],
                        op0=ALU.mult, op1=ALU.max)
                    EX = sp.tile([128, K, 4], BF16, tag="EX")
                    nc.scalar.activation(out=EX[:], in_=LK[:], func=AF.Exp)
                    rhs_w = sp.tile([128, K, 4 + XWC], BF16, tag="rhs1", bufs=1)
                    nc.scalar.activation(out=rhs_w[:, :, 0:4], in_=LK[:],
                                         func=AF.Exp)
                    if t == T - 1:
                        nc.vector.tensor_copy(
                            out=EXall[:, w * K:(w + 1) * K, :], in_=EX[:])
                    nc.vector.tensor_tensor(
                        out=rhs_w[:, :, 4:4 + XWC].rearrange(
                            "p k (h c) -> p k h c", h=HEADS),
                        in0=G[:, :, 0:XWC].rearrange(
                            "p k (h c) -> p k h c", h=HEADS),
                        in1=EX[:].unsqueeze(-1).to_broadcast(
                            (128, K, HEADS, C1)),
                        op=ALU.mult)
                    ps = spp.tile([128, 4 + XWC], F32, tag="agg")
                    for k in range(K):
                        nc.tensor.matmul(
                            ps[:], S_w[:, k * 128:(k + 1) * 128], rhs_w[:, k, :],
                            start=(k == 0), stop=(k == K - 1))
                    nc.scalar.copy(out=SUMS[:, w, :], in_=ps[:])
                node_phase(t, 1, sp, spp, SUMS, None)

            def gru_step(t, sp, spp):
                GI = sp.tile([128, NWL, 3 * HID], F32, tag="GI", bufs=1)
                GH = sp.tile([128, NWL, 3 * HID], F32, tag="GH", bufs=1)
                for w in range(NWL):
                    gi_ps = spp.tile([128, 3 * HID], F32, tag="psx")
                    nc.tensor.matmul(
                        gi_ps[:],
                        hsT_t[:, t * NL + w * 128: t * NL + (w + 1) * 128],
                        WihT_t[:], start=True, stop=True)
                    tp = spp.tile([HID, 128], F32, tag="tp")
                    nc.tensor.transpose(tp[:], hgru_t[:, w, :], ident_t[:])
                    hTt = sp.tile([HID, 128], F32, tag="hTt", bufs=1)
                    nc.vector.tensor_copy(out=hTt[:], in_=tp[:])
                    gh_ps = spp.tile([128, 3 * HID], F32, tag="mm2")
                    nc.tensor.matmul(gh_ps[:], hTt[:], WhhT_t[:],
                                     start=True, stop=True)
                    nc.scalar.copy(out=GI[:, w, :], in_=gi_ps[:])
                    nc.scalar.copy(out=GH[:, w, :], in_=gh_ps[:])
                nc.vector.tensor_tensor(
                    out=GI[:], in0=GI[:],
                    in1=bih_t[:].unsqueeze(1).to_broadcast((128, NWL, 3 * HID)),
                    op=ALU.add)
                nc.vector.tensor_tensor(
                    out=GH[:], in0=GH[:],
                    in1=bhh_t[:].unsqueeze(1).to_broadcast((128, NWL, 3 * HID)),
                    op=ALU.add)
                RZ = sp.tile([128, NWL, 2 * HID], F32, tag="RZ", bufs=1)
                nc.vector.tensor_tensor(out=RZ[:], in0=GI[:, :, 0:2 * HID],
                                        in1=GH[:, :, 0:2 * HID], op=ALU.add)
                nc.scalar.activation(out=RZ[:], in_=RZ[:], func=AF.Sigmoid)
                GG = sp.tile([128, NWL, HID], F32, tag="GG", bufs=1)
                nc.vector.tensor_tensor(out=GG[:], in0=RZ[:, :, 0:HID],
                                        in1=GH[:, :, 2 * HID:3 * HID],
                                        op=ALU.mult)
                nc.vector.tensor_tensor(out=GG[:], in0=GG[:],
                                        in1=GI[:, :, 2 * HID:3 * HID],
                                        op=ALU.add)
                nc.scalar.activation(out=GG[:], in_=GG[:], func=AF.Tanh)
                HD = sp.tile([128, NWL, HID], F32, tag="HD", bufs=1)
                nc.vector.tensor_tensor(out=HD[:], in0=hgru_t[:], in1=GG[:],
                                        op=ALU.subtract)
                nc.vector.tensor_tensor(out=HD[:], in0=HD[:],
                                        in1=RZ[:, :, HID:2 * HID], op=ALU.mult)
                nc.vector.tensor_tensor(out=hgru_t[:], in0=GG[:], in1=HD[:],
                                        op=ALU.add)

            with tc.tile_pool(name="sp", bufs=2) as sp, \
                 tc.tile_pool(name="spp", bufs=2, space="PSUM") as spp, \
                 tc.tile_pool(name="sdp", bufs=1) as sdp:
                nc.vector.memset(hgru_t[:], 0.0)
                for t in range(T):
                    # ---- layer-0 edges (gather-free) ----
                    h0T = sdp.tile([HID, NL], BF16, tag="h0T", name=f"h0T{t}")
                    gat0(t, sp, spp, h0T)
                    nc.sync.dma_start(out=h0T_bounce[:], in_=h0T[:])
                    nc.gpsimd.collective_compute(
                        "AllGather", ALU.bypass,
                        replica_groups=[list(range(NCORES))],
                        ins=[h0T_bounce[:]], outs=[h0T_full[:]])
                    # ---- xw1 table ----
                    GRP = 5
                    NWG = NG // 128
                    for g in range(NWG // GRP):
                        cw = (g * GRP) // NWL
                        hfg = sp.tile([HID, NL], BF16, tag="hfg", bufs=1)
                        nc.sync.dma_start(
                            out=hfg[:],
                            in_=h0T_full[cw * HID:(cw + 1) * HID, :])
                        xw1_sb = sp.tile([128, GRP, ROW1], BF16, tag="xw1g", bufs=1)
                        for j in range(GRP):
                            w = g * GRP + j
                            lw = w % NWL
                            ps = spp.tile([128, HEADS * C1 + 8], F32, tag="mm2")
                            nc.tensor.matmul(
                                ps[:], hfg[:, lw * 128:(lw + 1) * 128],
                                W1e_t[:], start=True, stop=True)
                            nc.vector.tensor_copy(
                                out=xw1_sb[:, j, 0:HEADS * C1 + 8], in_=ps[:])
                        nc.sync.dma_start(
                            out=xw1_tabs[t % 2][g * GRP * 128:(g + 1) * GRP * 128, :]
                            .rearrange("(w p) c -> p w c", p=128),
                            in_=xw1_sb[:])
                    # ---- layer-1 edges + LN ----
                    gat1(t, sp, spp)
                    gru_step(t, sp, spp)

            # ---------- alpha output (t = T-1, layer 1) ----------
            with tc.tile_pool(name="pal", bufs=2) as sp:
                for w in range(NWL):
                    R = sp.tile([128, K, 64], F32, tag="R")
                    c0 = 0
                    for si_, sbs in enumerate(SBS):
                        nc.gpsimd.dma_gather(
                            out_ap=R[:, c0:c0 + sbs, :], in_ap=r_tab[:],
                            idxs_ap=dstl_ts[w][si_][:],
                            num_idxs=sbs * 128, num_idxs_reg=sbs * 128,
                            elem_size=64, queue_num=(w * NSB + si_) % 4)
                        c0 += sbs
                    alw = sp.tile([128, K, 4], F32, tag="alw")
                    nc.vector.tensor_tensor(
                        out=alw[:], in0=EXall[:, w * K:(w + 1) * K, :],
                        in1=R[:, :, 0:4], op=ALU.mult)
                    nc.sync.dma_start(
                        out=D["alpha_out"][:, w * K * 4:(w + 1) * K * 4],
                        in_=alw[:])

            with tc.tile_pool(name="pg", bufs=1) as spg:
                ht_o = spg.tile([128, NWL, HID], F32, tag="hto")
                nc.vector.tensor_copy(out=ht_o[:], in_=hgru_t[:])
                nc.sync.dma_start(
                    out=D["hT_out"][:].rearrange("(w p) c -> p w c", p=128),
                    in_=ht_o[:])
    nc.compile()
    _split_waits(nc)
    return nc


_CACHE = {}


def _install_trace_hook():
    """Dev-only: shim the missing antenv.axon_hooks so run_bass_kernel_spmd
    trace=True can NTFF-profile through the axon .so; skip artifact upload."""
    import sys
    import types

    if "antenv.axon_hooks" not in sys.modules:
        mod = types.ModuleType("antenv.axon_hooks")
        mod._hook = None
        mod.set_axon_ntff_profile_hook = lambda h: setattr(mod, "_hook", h)
        mod.get_axon_ntff_profile_hook = lambda: mod._hook
        sys.modules["antenv.axon_hooks"] = mod
        try:
            import antenv
            antenv.axon_hooks = mod
        except ImportError:
            pass
        from trn_agent_boot.trn_boot import _ntff_profile_via_ctypes
        mod._hook = _ntff_profile_via_ctypes("/opt/axon/libaxon_pjrt.so")
    import concourse.bass_utils as bu
    bu.upload_artifacts = lambda tmpdir: tmpdir


def kernel(**inputs):
    global LAST_EXEC_NS
    prep = _host_prep(inputs)
    K = prep["K"]
    if K not in _CACHE:
        _CACHE[K] = _build(K)
    nc = _CACHE[K]
    in_maps = []
    for c in range(NCORES):
        m = dict(prep["consts"])
        m.update(prep["percore"][c])
        in_maps.append(m)
    trace = bool(int(os.environ.get("GAT_TRACE", "0")))
    if trace:
        _install_trace_hook()
    try:
        res = run_bass_kernel_spmd(
            nc, in_maps, list(range(NCORES)), trace=trace,
            tmpdir=os.environ.get("GAT_TRACE_DIR"))
    except Exception:
        if not trace:
            raise
        res = run_bass_kernel_spmd(nc, in_maps, list(range(NCORES)))
    LAST_EXEC_NS = res.exec_time_ns

    hT = np.empty((N, HID), np.float32)
    EP = prep["EP"]
    alphas = np.empty((EP, HEADS), np.float32)
    for c in range(NCORES):
        r = res.results[c]
        hT[c * NPC:(c + 1) * NPC] = r["hT_out"][:NPC]
        a = r["alpha_out"].reshape(128, -1, 4).transpose(1, 0, 2).reshape(-1, 4)
        eids, slots = prep["slot_of"][c]
        alphas[eids] = a[slots]
    return hT, alphas, prep["aei"]


# revision 28
# speedup vs baseline: 1.2154x; 1.2154x over previous
"""GAT temporal encoder (2-layer GAT x 6 timesteps + GRU) on 8 trn2 cores.

Sharding: 1D node partition (1250 nodes/core, padded to 1280 = 10 windows of
128), edges partitioned by destination, sorted by dst, padded so each window
owns exactly K chunks of 128 edges.

Layer 0 is gather-free on device: the host pre-gathers x[src]||x[dst] per
edge slot (feature-major, bf16), and one PE matmul per 128-edge chunk
against [[W0,V0s],[0,V0d]] yields both xw0[src] and the complete attention
logit. Layer 1 dma_gathers rows of the bf16 xw1||s_src||s_dst table by src
(4 SWDGE queues), with per-window s_dst broadcast to edges via a
host-precomputed one-hot S^T matmul. Segment softmax + aggregation run as
one-hot S matmuls accumulated in PSUM per chunk (normalization applied
after aggregation at the node level, which is algebraically identical).
h0 is all-gathered between layers; GRU/LN/node math are batched f32 ops.
"""
import os
import numpy as np
import ml_dtypes

import bass_rust
import concourse.bacc as bacc
import concourse.tile as tile
from concourse import mybir
from concourse.bass_utils import run_bass_kernel_spmd

AF = mybir.ActivationFunctionType
ALU = mybir.AluOpType
F32 = mybir.dt.float32
BF16 = mybir.dt.bfloat16
I16 = mybir.dt.int16
NPBF = ml_dtypes.bfloat16

NCORES = 8
N, T, F, HID, HEADS, E = 10000, 6, 64, 64, 4, 160000
NPC = N // NCORES               # 1250 nodes per core
NWL = (NPC + 127) // 128        # 10 local windows
NL = NWL * 128                  # 1280 padded local nodes
NG = NCORES * NL                # 10240 padded global nodes
C0 = HID // HEADS               # 16 (layer-0 head dim)
C1 = HID                        # 64 (layer-1 head dim)
ROW1 = 384                      # bf16 cols: xw(256) s_src(4) s_dst(4) pad

LAST_EXEC_NS = None


def _split_waits(nc, maxw=1):
    """This container's walrus allows very few sync-wait slots per
    instruction; move excess waits onto preceding same-engine nops."""
    nsplit = 0
    for func in nc.m.functions:
        for bb in func.blocks:
            new = []
            for ins in bb.instructions:
                si = ins.sync_info
                waits = list(si.on_wait) if si is not None and si.on_wait else []
                if len(waits) > maxw:
                    nsplit += 1
                    rest = waits[maxw:]
                    for i in range(0, len(rest), maxw):
                        nop = mybir.InstNoOp(
                            name=f"I-wsplit-{nsplit}-{i}", ins=[], outs=[]
                        )
                        nop.engine = ins.engine
                        nop.sync_info = bass_rust.SyncInfo(
                            on_wait=rest[i : i + maxw], on_update=[]
                        )
                        new.append(nop)
                    si.on_wait = waits[:maxw]
                new.append(ins)
            bb.instructions = new
    return nsplit


def _wrap16(a):
    """flat [n] -> replicated-wrapped [128, n/16] int16 (flat k at [k%16, k//16],
    replicated across the 8 gpsimd cores' partition groups)."""
    n = a.shape[0]
    assert n % 16 == 0
    return np.tile(a.reshape(n // 16, 16).T, (8, 1)).astype(np.int16).copy()


def _host_prep(inputs):
    x = np.asarray(inputs["x"], np.float32)
    ei = np.asarray(inputs["edge_index"])
    W0 = np.asarray(inputs["W0"], np.float32)
    a_src0 = np.asarray(inputs["a_src0"], np.float32)
    a_dst0 = np.asarray(inputs["a_dst0"], np.float32)
    W1 = np.asarray(inputs["W1"], np.float32)
    a_src1 = np.asarray(inputs["a_src1"], np.float32)
    a_dst1 = np.asarray(inputs["a_dst1"], np.float32)

    loops = np.arange(N, dtype=ei.dtype)
    src = np.concatenate([ei[0], loops])
    dst = np.concatenate([ei[1], loops])
    EP = src.shape[0]  # E + N

    core_of = dst // NPC
    per_core = []
    Kmax = 1
    for c in range(NCORES):
        sel = np.nonzero(core_of == c)[0]
        order = np.argsort(dst[sel], kind="stable")
        eids = sel[order]
        dl = (dst[eids] - c * NPC).astype(np.int64)
        w_of = dl // 128
        cnt = np.bincount(w_of, minlength=NWL)
        Kmax = max(Kmax, int(np.ceil(cnt.max() / 128)))
        per_core.append((eids, dl))

    K = Kmax
    NCH = NWL * K
    EB = NCH * 128
    SBS = [8] * (K // 8) + ([K % 8] if K % 8 else [])

    xbf = x.astype(NPBF)        # [N, T, F]
    percore = []
    slot_of = []
    for c in range(NCORES):
        eids, dl = per_core[c]
        srcg = src[eids]
        srcp = ((srcg // NPC) * NL + (srcg % NPC)).astype(np.int64)
        s_src = np.zeros(EB, np.int64)      # padded-global src per slot
        s_orig = np.zeros(EB, np.int64)     # original src node id (pad->0)
        d_orig = np.zeros(EB, np.int64)     # original dst node id (pad->0)
        s_dstw = np.full(EB, -1, np.int64)  # dst-in-window; -1 => pad edge
        slots = np.empty(len(eids), np.int64)
        w_of = dl // 128
        for w in range(NWL):
            idxs = np.nonzero(w_of == w)[0]
            base = w * K * 128
            s_src[base : base + len(idxs)] = srcp[idxs]
            s_orig[base : base + len(idxs)] = srcg[idxs]
            d_orig[base : base + len(idxs)] = c * NPC + dl[idxs]
            s_dstw[base : base + len(idxs)] = dl[idxs] - w * 128
            slots[idxs] = base + np.arange(len(idxs))

        m = {}
        for si_, sbs in enumerate(SBS):
            arr = np.zeros((NWL, 128, sbs * 8), np.int16)
            arl = np.zeros((NWL, 128, sbs * 8), np.int16)
            c0 = sum(SBS[:si_])
            for w in range(NWL):
                lo = (w * K + c0) * 128
                hi = lo + sbs * 128
                arr[w] = _wrap16(s_src[lo:hi])
                dloc = s_dstw[lo:hi].copy()
                dloc[dloc >= 0] += w * 128
                dloc[dloc < 0] = 0
                arl[w] = _wrap16(dloc)
            m[f"srcp{si_}"] = arr
            m[f"dstl{si_}"] = arl
        # one-hot S / S^T per window (bf16)
        dw = s_dstw.reshape(NWL, K, 128)
        S = np.zeros((NWL, 128, K * 128), NPBF)
        St = np.zeros((NWL, 128, K * 128), NPBF)
        for w in range(NWL):
            for k in range(K):
                d = dw[w, k]
                p = np.nonzero(d >= 0)[0]
                S[w, p, k * 128 + d[p]] = 1
                St[w, d[p], k * 128 + p] = 1
        m["S"] = S
        m["St"] = St
        # host-pre-gathered per-edge x: rows 0:64 = x[src]^T, 64:128 = x[dst]^T
        xc = np.empty((T, 128, EB), NPBF)
        xe = xbf[s_orig]            # [EB, T, F]
        xd = xbf[d_orig]
        xc[:, 0:F, :] = xe.transpose(1, 2, 0)
        xc[:, F:2 * F, :] = xd.transpose(1, 2, 0)
        m["xcomb"] = xc
        ln_ = c * NL + np.arange(NL)
        m["locg0"] = _wrap16(ln_[:NL // 2].astype(np.int64))
        m["locg1"] = _wrap16(ln_[NL // 2:].astype(np.int64))
        percore.append(m)
        slot_of.append((eids, slots))

    V0s = np.einsum("khc,hc->kh", W0.reshape(F, HEADS, C0), a_src0)
    V0d = np.einsum("khc,hc->kh", W0.reshape(F, HEADS, C0), a_dst0)
    # [[W0, V0s], [0, V0d]]: [128, 68]
    W0ee = np.zeros((2 * F, HID + 4), np.float32)
    W0ee[0:F, 0:HID] = W0
    W0ee[0:F, HID:HID + 4] = V0s
    W0ee[F:2 * F, HID:HID + 4] = V0d
    V1s = np.einsum("khc,hc->kh", W1.reshape(HID, HEADS, C1), a_src1)
    V1d = np.einsum("khc,hc->kh", W1.reshape(HID, HEADS, C1), a_dst1)
    W1e = np.concatenate([W1, V1s, V1d], axis=1).astype(NPBF)

    rep = lambda v, n: np.broadcast_to(
        np.asarray(v, np.float32).reshape(1, n), (128, n)).copy()
    consts = {
        "W0ee": W0ee.astype(NPBF), "W1e": W1e,
        "b0": rep(inputs["b0"], HID), "b1": rep(inputs["b1"], HID),
        "lng": rep(inputs["ln_g"], HID), "lnb": rep(inputs["ln_b"], HID),
        "WihT": np.asarray(inputs["W_ih"], np.float32).T.copy(),
        "WhhT": np.asarray(inputs["W_hh"], np.float32).T.copy(),
        "bih": rep(inputs["b_ih"], 3 * HID), "bhh": rep(inputs["b_hh"], 3 * HID),
        "ident": np.eye(128, dtype=np.float32),
    }
    aei = np.stack([src, dst]).astype(ei.dtype)
    return dict(K=K, NCH=NCH, EB=EB, consts=consts, percore=percore,
                slot_of=slot_of, aei=aei, EP=EP)


def _build(K):
    NCH = NWL * K
    EB = NCH * 128
    SBS = [8] * (K // 8) + ([K % 8] if K % 8 else [])
    NSB = len(SBS)
    nc = bacc.Bacc(None, num_swdge_queues=4)
    D = {}

    def par(name, shape, dt=F32, out=False):
        D[name] = nc.declare_dram_parameter(name, list(shape), dt, isOutput=out)
        return D[name]

    par("xcomb", [T, 128, EB], BF16)
    par("W0ee", [2 * F, HID + 4], BF16)
    par("W1e", [HID, HEADS * C1 + 8], BF16)
    par("b0", [128, HID]); par("b1", [128, HID])
    par("lng", [128, HID]); par("lnb", [128, HID])
    par("WihT", [HID, 3 * HID]); par("WhhT", [HID, 3 * HID])
    par("bih", [128, 3 * HID]); par("bhh", [128, 3 * HID])
    par("ident", [128, 128])
    for si_, sbs in enumerate(SBS):
        par(f"srcp{si_}", [NWL, 128, sbs * 8], I16)
        par(f"dstl{si_}", [NWL, 128, sbs * 8], I16)
    par("S", [NWL, 128, K * 128], BF16)
    par("St", [NWL, 128, K * 128], BF16)
    par("locg0", [128, NL // 32], I16)
    par("locg1", [128, NL // 32], I16)
    par("hT_out", [NL, HID], out=True)
    par("alpha_out", [128, NCH * 4], out=True)

    xw1_tabs = [nc.dram_tensor(f"xw1_{i}", [NG, ROW1], BF16) for i in range(2)]
    r_tab = nc.dram_tensor("r_tab", [NL, HID], F32)
    h0T_bounce = nc.dram_tensor("h0T_bounce", [HID, NL], BF16)
    h0T_full = nc.dram_tensor("h0T_full", [NCORES * HID, NL], BF16,
                              addr_space="Shared")

    with tile.TileContext(nc) as tc:
        with tc.tile_pool(name="persist", bufs=1) as pp:
            W0ee_t = pp.tile([2 * F, HID + 4], BF16)
            W1e_t = pp.tile([HID, HEADS * C1 + 8], BF16)
            b0_t = pp.tile([128, HID], F32, tag="b0t")
            b1_t = pp.tile([128, HID], F32, tag="b1t")
            lng_t = pp.tile([128, HID], F32, tag="lngt")
            lnb_t = pp.tile([128, HID], F32, tag="lnbt")
            WihT_t = pp.tile([HID, 3 * HID], F32, tag="wiht")
            WhhT_t = pp.tile([HID, 3 * HID], F32, tag="whht")
            bih_t = pp.tile([128, 3 * HID], F32, tag="biht")
            bhh_t = pp.tile([128, 3 * HID], F32, tag="bhht")
            ident_t = pp.tile([128, 128], F32, tag="identt")
            locg0_t = pp.tile([128, NL // 32], I16, tag="locg0t")
            locg1_t = pp.tile([128, NL // 32], I16, tag="locg1t")
            eps_t = pp.tile([128, 1], F32, tag="epst")
            nc.vector.memset(eps_t[:], 1e-5)
            for nm, tl in [("W0ee", W0ee_t), ("W1e", W1e_t), ("b0", b0_t),
                           ("b1", b1_t), ("lng", lng_t), ("lnb", lnb_t),
                           ("WihT", WihT_t), ("WhhT", WhhT_t), ("bih", bih_t),
                           ("bhh", bhh_t), ("ident", ident_t),
                           ("locg0", locg0_t), ("locg1", locg1_t)]:
                nc.sync.dma_start(out=tl[:], in_=D[nm][:])
            srcp_ts, dstl_ts = [], []
            for w in range(NWL):
                rowt = []
                for si_, sbs in enumerate(SBS):
                    tl = pp.tile([128, sbs * 8], I16, tag=f"srcp{w}_{si_}",
                                 name=f"srcp{w}_{si_}")
                    nc.sync.dma_start(out=tl[:], in_=D[f"srcp{si_}"][w])
                    rowt.append(tl)
                srcp_ts.append(rowt)
                rowt = []
                for si_, sbs in enumerate(SBS):
                    tl = pp.tile([128, sbs * 8], I16, tag=f"dstl{w}_{si_}",
                                 name=f"dstl{w}_{si_}")
                    nc.sync.dma_start(out=tl[:], in_=D[f"dstl{si_}"][w])
                    rowt.append(tl)
                dstl_ts.append(rowt)

            EXall = pp.tile([128, NCH, 4], F32)        # t5 L1 ex (for alpha)
            hsT_t = pp.tile([HID, T * NL], F32)        # LN outputs, transposed
            hgru_t = pp.tile([128, NWL, HID], F32, tag="hgrut")

            def node_phase(t, lay, sp, spp, SUMS, h0T):
                """batched across windows; SUMS [128, NWL, 4+XWC] f32."""
                XWC = HID if lay == 0 else HEADS * C1
                CC = C0 if lay == 0 else C1
                R4 = sp.tile([128, NWL, 4], F32, tag="R4")
                nc.vector.tensor_scalar(out=R4[:], in0=SUMS[:, :, 0:4],
                                        scalar1=1e-16, scalar2=None, op0=ALU.add)
                nc.vector.reciprocal(out=R4[:], in_=R4[:])
                if lay == 1 and t == T - 1:
                    RT = sp.tile([128, NWL, HID], F32, tag="RT")
                    nc.vector.memset(RT[:], 0.0)
                    nc.vector.tensor_copy(out=RT[:, :, 0:4], in_=R4[:])
                    nc.sync.dma_start(
                        out=r_tab[:].rearrange("(w p) c -> p w c", p=128),
                        in_=RT[:])
                ON = sp.tile([128, NWL, XWC], F32, tag=f"ON{lay}", bufs=(2 if lay == 0 else 1))
                nc.vector.tensor_tensor(
                    out=ON[:].rearrange("p w (h c) -> p w h c", h=HEADS),
                    in0=SUMS[:, :, 4:4 + XWC].rearrange(
                        "p w (h c) -> p w h c", h=HEADS),
                    in1=R4[:].unsqueeze(-1).to_broadcast((128, NWL, HEADS, CC)),
                    op=ALU.mult)
                if lay == 1:
                    # head mean -> HM [128, NWL, 64]
                    HM = sp.tile([128, NWL, C1], F32, tag="HM")
                    nc.vector.tensor_tensor(out=HM[:], in0=ON[:, :, 0:C1],
                                            in1=ON[:, :, C1:2 * C1], op=ALU.add)
                    nc.vector.tensor_tensor(out=HM[:], in0=HM[:],
                                            in1=ON[:, :, 2 * C1:3 * C1], op=ALU.add)
                    nc.vector.tensor_tensor(out=HM[:], in0=HM[:],
                                            in1=ON[:, :, 3 * C1:4 * C1], op=ALU.add)
                    nc.vector.tensor_scalar(out=HM[:], in0=HM[:], scalar1=0.25,
                                            scalar2=None, op0=ALU.mult)
                    ON = HM
                bias = b0_t if lay == 0 else b1_t
                nc.vector.tensor_tensor(
                    out=ON[:, :, 0:HID], in0=ON[:, :, 0:HID],
                    in1=bias[:].unsqueeze(1).to_broadcast((128, NWL, HID)),
                    op=ALU.add)
                # elu in place
                e1 = sp.tile([128, NWL, HID], F32, tag="elu1", bufs=1)
                nc.vector.tensor_scalar(out=e1[:], in0=ON[:, :, 0:HID],
                                        scalar1=0.0, scalar2=None, op0=ALU.min)
                nc.scalar.activation(out=e1[:], in_=e1[:], func=AF.Exp)
                e2 = sp.tile([128, NWL, HID], F32, tag="elu2", bufs=1)
                nc.scalar.activation(out=e2[:], in_=ON[:, :, 0:HID], func=AF.Relu)
                nc.vector.scalar_tensor_tensor(
                    out=ON[:, :, 0:HID], in0=e1[:], scalar=-1.0, in1=e2[:],
                    op0=ALU.add, op1=ALU.add)
                if lay == 0:
                    for w in range(NWL):
                        tp = spp.tile([HID, 128], F32, tag="tp", bufs=1)
                        nc.tensor.transpose(tp[:], ON[:, w, 0:HID], ident_t[:])
                        nc.vector.tensor_copy(out=h0T[:, w * 128:(w + 1) * 128],
                                              in_=tp[:])
                    return
                # LayerNorm batched over windows
                MU = sp.tile([128, NWL], F32, tag="MU")
                nc.vector.tensor_reduce(out=MU[:], in_=ON[:],
                                        axis=mybir.AxisListType.X, op=ALU.add)
                nc.vector.tensor_scalar(out=MU[:], in0=MU[:], scalar1=1.0 / HID,
                                        scalar2=None, op0=ALU.mult)
                XC = sp.tile([128, NWL, HID], F32, tag="XC", bufs=1)
                nc.vector.tensor_tensor(
                    out=XC[:], in0=ON[:],
                    in1=MU[:].unsqueeze(-1).to_broadcast((128, NWL, HID)),
                    op=ALU.subtract)
                SQ = sp.tile([128, NWL, HID], F32, tag="SQ", bufs=1)
                nc.vector.tensor_tensor(out=SQ[:], in0=XC[:], in1=XC[:],
                                        op=ALU.mult)
                VV = sp.tile([128, NWL], F32, tag="VV")
                nc.vector.tensor_reduce(out=VV[:], in_=SQ[:],
                                        axis=mybir.AxisListType.X, op=ALU.add)
                nc.vector.tensor_scalar(out=VV[:], in0=VV[:], scalar1=1.0 / HID,
                                        scalar2=1e-5, op0=ALU.mult, op1=ALU.add)
                nc.scalar.activation(out=VV[:], in_=VV[:], func=AF.Sqrt)
                nc.vector.reciprocal(out=VV[:], in_=VV[:])
                nc.vector.tensor_tensor(
                    out=XC[:], in0=XC[:],
                    in1=VV[:].unsqueeze(-1).to_broadcast((128, NWL, HID)),
                    op=ALU.mult)
                nc.vector.tensor_tensor(
                    out=XC[:], in0=XC[:],
                    in1=lng_t[:].unsqueeze(1).to_broadcast((128, NWL, HID)),
                    op=ALU.mult)
                nc.vector.tensor_tensor(
                    out=XC[:], in0=XC[:],
                    in1=lnb_t[:].unsqueeze(1).to_broadcast((128, NWL, HID)),
                    op=ALU.add)
                for w in range(NWL):
                    tp = spp.tile([HID, 128], F32, tag="tp", bufs=1)
                    nc.tensor.transpose(tp[:], XC[:, w, :], ident_t[:])
                    nc.vector.tensor_copy(
                        out=hsT_t[:, t * NL + w * 128: t * NL + (w + 1) * 128],
                        in_=tp[:])

            def gat0(t, sp, spp, h0T):
                XWC = HID
                SUMS = sp.tile([128, NWL, 4 + XWC], F32, tag="SUMS0")
                for w in range(NWL):
                    xct = sp.tile([128, K * 128], BF16, tag="xct")
                    nc.sync.dma_start(
                        out=xct[:],
                        in_=D["xcomb"][t][:, w * K * 128:(w + 1) * K * 128])
                    S_w = sp.tile([128, K * 128], BF16, tag="S_w")
                    nc.sync.dma_start(out=S_w[:], in_=D["S"][w])
                    Ge = sp.tile([128, K, 4 + XWC], BF16, tag="Ge")
                    for k in range(K):
                        psx = spp.tile([128, HID + 4], F32, tag="psx", bufs=3)
                        nc.tensor.matmul(
                            psx[:], xct[:, k * 128:(k + 1) * 128], W0ee_t[:],
                            start=True, stop=True)
                        nc.scalar.copy(out=Ge[:, k, :], in_=psx[:])
                    rhs_w = sp.tile([128, K, 4 + XWC], BF16, tag="rhs0")
                    LK = sp.tile([128, K, 4], BF16, tag="LK")
                    nc.vector.scalar_tensor_tensor(
                        out=LK[:], in0=Ge[:, :, XWC:XWC + 4], scalar=0.2,
                        in1=Ge[:, :, XWC:XWC + 4], op0=ALU.mult, op1=ALU.max)
                    EX = sp.tile([128, K, 4], BF16, tag="EX")
                    nc.scalar.activation(out=EX[:], in_=LK[:], func=AF.Exp)
                    nc.scalar.activation(out=rhs_w[:, :, 0:4], in_=LK[:],
                                         func=AF.Exp)
                    nc.vector.tensor_tensor(
                        out=rhs_w[:, :, 4:4 + XWC].rearrange(
                            "p k (h c) -> p k h c", h=HEADS),
                        in0=Ge[:, :, 0:XWC].rearrange(
                            "p k (h c) -> p k h c", h=HEADS),
                        in1=EX[:].unsqueeze(-1).to_broadcast(
                            (128, K, HEADS, C0)),
                        op=ALU.mult)
                    ps = spp.tile([128, 4 + XWC], F32, tag="agg")
                    for k in range(K):
                        nc.tensor.matmul(
                            ps[:], S_w[:, k * 128:(k + 1) * 128], rhs_w[:, k, :],
                            start=(k == 0), stop=(k == K - 1))
                    nc.scalar.copy(out=SUMS[:, w, :], in_=ps[:])
                node_phase(t, 0, sp, spp, SUMS, h0T)

            def gat1(t, sp, spp):
                XWC = HEADS * C1
                tab = xw1_tabs[t % 2]
                sdAll = sp.tile([128, NWL, 128], BF16, tag="sdAll")
                for g, lg in enumerate((locg0_t, locg1_t)):
                    nc.gpsimd.dma_gather(
                        out_ap=sdAll[:, g * 5:(g + 1) * 5, :],
                        in_ap=tab[:, ROW1 - 128:ROW1],
                        idxs_ap=lg[:],
                        num_idxs=640, num_idxs_reg=640,
                        elem_size=128, elem_step=ROW1, queue_num=3)
                SUMS = sp.tile([128, NWL, 4 + XWC], F32, tag="SUMS1", bufs=1)
                for w in range(NWL):
                    G = sp.tile([128, K, ROW1], BF16, tag="G1", bufs=2)
                    c0 = 0
                    for si_, sbs in enumerate(SBS):
                        nc.gpsimd.dma_gather(
                            out_ap=G[:, c0:c0 + sbs, :], in_ap=tab[:],
                            idxs_ap=srcp_ts[w][si_][:],
                            num_idxs=sbs * 128, num_idxs_reg=sbs * 128,
                            elem_size=ROW1, queue_num=(w * NSB + si_) % 3)
                        c0 += sbs
                    S_w = sp.tile([128, K * 128], BF16, tag="S_w")
                    nc.sync.dma_start(out=S_w[:], in_=D["S"][w])
                    St_w = sp.tile([128, K * 128], BF16, tag="St_w")
                    nc.sync.dma_start(out=St_w[:], in_=D["St"][w])
                    ps2 = spp.tile([128, K, 4], F32, tag="mm2")
                    for k in range(K):
                        nc.tensor.matmul(
                            ps2[:, k, :], St_w[:, k * 128:(k + 1) * 128],
                            sdAll[:, w, 4:8], start=True, stop=True)
                    AL = sp.tile([128, K, 4], BF16, tag="AL")
                    nc.vector.tensor_tensor(
                        out=AL[:], in0=G[:, :, XWC:XWC + 4], in1=ps2[:],
                        op=ALU.add)
                    LK = sp.tile([128, K, 4], BF16, tag="LK")
                    nc.vector.scalar_tensor_tensor(
                        out=LK[:], in0=AL[:], scalar=0.2, in1=AL[:],
                        op0=ALU.mult, op1=ALU.max)
                    EX = sp.tile([128, K, 4], BF16, tag="EX")
                    nc.scalar.activation(out=EX[:], in_=LK[:], func=AF.Exp)
                    rhs_w = sp.tile([128, K, 4 + XWC], BF16, tag="rhs1", bufs=1)
                    nc.scalar.activation(out=rhs_w[:, :, 0:4], in_=LK[:],
                                         func=AF.Exp)
                    if t == T - 1:
                        nc.vector.tensor_copy(
                            out=EXall[:, w * K:(w + 1) * K, :], in_=EX[:])
                    nc.vector.tensor_tensor(
                        out=rhs_w[:, :, 4:4 + XWC].rearrange(
                            "p k (h c) -> p k h c", h=HEADS),
                        in0=G[:, :, 0:XWC].rearrange(
                            "p k (h c) -> p k h c", h=HEADS),
                        in1=EX[:].unsqueeze(-1).to_broadcast(
                            (128, K, HEADS, C1)),
                        op=ALU.mult)
                    ps = spp.tile([128, 4 + XWC], F32, tag="agg")
                    for k in range(K):
                        nc.tensor.matmul(
                            ps[:], S_w[:, k * 128:(k + 1) * 128], rhs_w[:, k, :],
                            start=(k == 0), stop=(k == K - 1))
                    nc.scalar.copy(out=SUMS[:, w, :], in_=ps[:])
                node_phase(t, 1, sp, spp, SUMS, None)

            with tc.tile_pool(name="sp", bufs=2) as sp, \
                 tc.tile_pool(name="spp", bufs=2, space="PSUM") as spp, \
                 tc.tile_pool(name="sdp", bufs=2) as sdp:
                for t in range(T):
                    # ---- layer-0 edges (gather-free) ----
                    h0T = sdp.tile([HID, NL], BF16, tag="h0T", name=f"h0T{t}")
                    gat0(t, sp, spp, h0T)
                    nc.sync.dma_start(out=h0T_bounce[:], in_=h0T[:])
                    nc.gpsimd.collective_compute(
                        "AllGather", ALU.bypass,
                        replica_groups=[list(range(NCORES))],
                        ins=[h0T_bounce[:]], outs=[h0T_full[:]])
                    # ---- xw1 table ----
                    GRP = 5
                    NWG = NG // 128
                    for g in range(NWG // GRP):
                        cw = (g * GRP) // NWL
                        hfg = sp.tile([HID, NL], BF16, tag="hfg")
                        nc.sync.dma_start(
                            out=hfg[:],
                            in_=h0T_full[cw * HID:(cw + 1) * HID, :])
                        xw1_sb = sp.tile([128, GRP, ROW1], BF16, tag="xw1g")
                        for j in range(GRP):
                            w = g * GRP + j
                            lw = w % NWL
                            ps = spp.tile([128, HEADS * C1 + 8], F32, tag="mm2")
                            nc.tensor.matmul(
                                ps[:], hfg[:, lw * 128:(lw + 1) * 128],
                                W1e_t[:], start=True, stop=True)
                            nc.vector.tensor_copy(
                                out=xw1_sb[:, j, 0:HEADS * C1 + 8], in_=ps[:])
                        nc.sync.dma_start(
                            out=xw1_tabs[t % 2][g * GRP * 128:(g + 1) * GRP * 128, :]
                            .rearrange("(w p) c -> p w c", p=128),
                            in_=xw1_sb[:])
                    # ---- layer-1 edges + LN ----
                    gat1(t, sp, spp)

            # ---------- alpha output (t = T-1, layer 1) ----------
            with tc.tile_pool(name="pal", bufs=2) as sp:
                for w in range(NWL):
                    R = sp.tile([128, K, 64], F32, tag="R")
                    c0 = 0
                    for si_, sbs in enumerate(SBS):
                        nc.gpsimd.dma_gather(
                            out_ap=R[:, c0:c0 + sbs, :], in_ap=r_tab[:],
                            idxs_ap=dstl_ts[w][si_][:],
                            num_idxs=sbs * 128, num_idxs_reg=sbs * 128,
                            elem_size=64, queue_num=(w * NSB + si_) % 4)
                        c0 += sbs
                    alw = sp.tile([128, K, 4], F32, tag="alw")
                    nc.vector.tensor_tensor(
                        out=alw[:], in0=EXall[:, w * K:(w + 1) * K, :],
                        in1=R[:, :, 0:4], op=ALU.mult)
                    nc.sync.dma_start(
                        out=D["alpha_out"][:, w * K * 4:(w + 1) * K * 4],
                        in_=alw[:])

            # ---------- GRU (f32, batched gate math) ----------
            with tc.tile_pool(name="pg", bufs=3) as sp, \
                 tc.tile_pool(name="pgp", bufs=2, space="PSUM") as spp:
                nc.vector.memset(hgru_t[:], 0.0)
                for t in range(T):
                    GI = sp.tile([128, NWL, 3 * HID], F32, tag="GI")
                    GH = sp.tile([128, NWL, 3 * HID], F32, tag="GH")
                    for w in range(NWL):
                        gi_ps = spp.tile([128, 3 * HID], F32, tag="gi")
                        nc.tensor.matmul(
                            gi_ps[:],
                            hsT_t[:, t * NL + w * 128: t * NL + (w + 1) * 128],
                            WihT_t[:], start=True, stop=True)
                        tp = spp.tile([HID, 128], F32, tag="tpg")
                        nc.tensor.transpose(tp[:], hgru_t[:, w, :], ident_t[:])
                        hTt = sp.tile([HID, 128], F32, tag="hTt")
                        nc.vector.tensor_copy(out=hTt[:], in_=tp[:])
                        gh_ps = spp.tile([128, 3 * HID], F32, tag="gh")
                        nc.tensor.matmul(gh_ps[:], hTt[:], WhhT_t[:],
                                         start=True, stop=True)
                        nc.scalar.copy(out=GI[:, w, :], in_=gi_ps[:])
                        nc.scalar.copy(out=GH[:, w, :], in_=gh_ps[:])
                    nc.vector.tensor_tensor(
                        out=GI[:], in0=GI[:],
                        in1=bih_t[:].unsqueeze(1).to_broadcast((128, NWL, 3 * HID)),
                        op=ALU.add)
                    nc.vector.tensor_tensor(
                        out=GH[:], in0=GH[:],
                        in1=bhh_t[:].unsqueeze(1).to_broadcast((128, NWL, 3 * HID)),
                        op=ALU.add)
                    RZ = sp.tile([128, NWL, 2 * HID], F32, tag="RZ")
                    nc.vector.tensor_tensor(out=RZ[:], in0=GI[:, :, 0:2 * HID],
                                            in1=GH[:, :, 0:2 * HID], op=ALU.add)
                    nc.scalar.activation(out=RZ[:], in_=RZ[:], func=AF.Sigmoid)
                    GG = sp.tile([128, NWL, HID], F32, tag="GG")
                    nc.vector.tensor_tensor(out=GG[:], in0=RZ[:, :, 0:HID],
                                            in1=GH[:, :, 2 * HID:3 * HID],
                                            op=ALU.mult)
                    nc.vector.tensor_tensor(out=GG[:], in0=GG[:],
                                            in1=GI[:, :, 2 * HID:3 * HID],
                                            op=ALU.add)
                    nc.scalar.activation(out=GG[:], in_=GG[:], func=AF.Tanh)
                    HD = sp.tile([128, NWL, HID], F32, tag="HD")
                    nc.vector.tensor_tensor(out=HD[:], in0=hgru_t[:], in1=GG[:],
                                            op=ALU.subtract)
                    nc.vector.tensor_tensor(out=HD[:], in0=HD[:],
                                            in1=RZ[:, :, HID:2 * HID],
                                            op=ALU.mult)
                    nc.vector.tensor_tensor(out=hgru_t[:], in0=GG[:], in1=HD[:],
                                            op=ALU.add)
                nc.sync.dma_start(
                    out=D["hT_out"][:].rearrange("(w p) c -> p w c", p=128),
                    in_=hgru_t[:])
    nc.compile()
    _split_waits(nc)
    return nc


_CACHE = {}


def _install_trace_hook():
    """Dev-only: shim the missing antenv.axon_hooks so run_bass_kernel_spmd
    trace=True can NTFF-profile through the axon .so; skip artifact upload."""
    import sys
    import types

    if "antenv.axon_hooks" not in sys.modules:
        mod = types.ModuleType("antenv.axon_hooks")
        mod._hook = None
        mod.set_axon_ntff_profile_hook = lambda h: setattr(mod, "_hook", h)
        mod.get_axon_ntff_profile_hook = lambda: mod._hook
        sys.modules["antenv.axon_hooks"] = mod
        try:
            import antenv
            antenv.axon_hooks = mod
        except ImportError:
            pass
        from trn_agent_boot.trn_boot import _ntff_profile_via_ctypes
        mod._hook = _ntff_profile_via_ctypes("/opt/axon/libaxon_pjrt.so")
    import concourse.bass_utils as bu
    bu.upload_artifacts = lambda tmpdir: tmpdir


def kernel(**inputs):
    global LAST_EXEC_NS
    prep = _host_prep(inputs)
    K = prep["K"]
    if K not in _CACHE:
        _CACHE[K] = _build(K)
    nc = _CACHE[K]
    in_maps = []
    for c in range(NCORES):
        m = dict(prep["consts"])
        m.update(prep["percore"][c])
        in_maps.append(m)
    trace = bool(int(os.environ.get("GAT_TRACE", "0")))
    if trace:
        _install_trace_hook()
    try:
        res = run_bass_kernel_spmd(
            nc, in_maps, list(range(NCORES)), trace=trace,
            tmpdir=os.environ.get("GAT_TRACE_DIR"))
    except Exception:
        if not trace:
            raise
        res = run_bass_kernel_spmd(nc, in_maps, list(range(NCORES)))
    LAST_EXEC_NS = res.exec_time_ns

    hT = np.empty((N, HID), np.float32)
    EP = prep["EP"]
    alphas = np.empty((EP, HEADS), np.float32)
    for c in range(NCORES):
        r = res.results[c]
        hT[c * NPC:(c + 1) * NPC] = r["hT_out"][:NPC]
        a = r["alpha_out"].reshape(128, -1, 4).transpose(1, 0, 2).reshape(-1, 4)
        eids, slots = prep["slot_of"][c]
        alphas[eids] = a[slots]
    return hT, alphas, prep["aei"]
